# revision 1
# baseline (speedup 1.0000x reference)
"""Trainium2 Bass kernel for nn_AppearanceComposability (sparse_attention).

Reference semantics, per (b, c) with 64x64 images, 3x3 unfold (pad 1):
  out_flat[m] = K_flat[m] * qv[m // 9],   qv[i] = Q_flat[9*i + 4]
where K_flat / Q_flat are the per-channel flattened unfold blocks
(kk*4096 + l, channel order (C, kh, kw)).

v2 implementation (bf16 end-to-end; rel err ~2.9e-3 vs gate 2e-2):
  - all tensors bf16: halves both DMA traffic and enables DVE 2x packing
  - chunks are processed in merged PAIR units (16KB DRAM row segments per
    store, vs 8KB singles: ~25% better effective DMA rate)
  - most chunks are "stretched": ACT pre-builds qs[l] = qv[(s+l)//9] by
    broadcast-copy straight out of the padded query image (runs at ~1
    elem/cyc; the 9x fan-out amortizes the strided read), then DVE does a
    flat contiguous bf16 tensor_tensor multiply which packs 2 elem/cyc
  - a few chunks stay "broadcast" on DVE (classic step-0 qv operand, 1
    elem/cyc) to balance the two engines' load
  - query-wrap zeros: for stretched chunks, 9-wide runs zeroed on the qs
    tile (DVE memset); for broadcast chunks, qv edge memsets as before
  - key-wrap zeros (x-edge columns): stride-64 memsets on the out tile
  - data parallel over batch: 8 cores, core b handles batch b
"""
import os
import sys

import numpy as np


def _ensure_path():
    try:
        import concourse  # noqa: F401
    except ImportError:
        for p in ("/opt/trn_rl_repo", "/root/.axon_site/_ro/trn_rl_repo"):
            if os.path.isdir(p):
                sys.path.insert(0, p)
                return


_ensure_path()

import concourse.bacc as bacc  # noqa: E402
import concourse.tile as tile  # noqa: E402
from concourse import mybir  # noqa: E402
from concourse.bass_utils import run_bass_kernel_spmd  # noqa: E402
from concourse.tile import add_dep_helper  # noqa: E402


def _install_ntff_hook_shim():
    """Provide antenv.axon_hooks when the image's antenv lacks it."""
    try:
        import antenv.axon_hooks  # noqa: F401
        return
    except ImportError:
        pass
    try:
        import types

        import antenv
        holder = {"hook": None, "tried": False}

        def set_axon_ntff_profile_hook(h):
            holder["hook"] = h
            holder["tried"] = True

        def get_axon_ntff_profile_hook():
            if not holder["tried"]:
                holder["tried"] = True
                try:
                    from trn_agent_boot.trn_boot import _ntff_profile_via_ctypes
                    so = "/opt/axon/libaxon_pjrt.so"
                    if os.path.exists(so):
                        holder["hook"] = _ntff_profile_via_ctypes(so)
                except Exception:
                    holder["hook"] = None
            return holder["hook"]

        mod = types.ModuleType("antenv.axon_hooks")
        mod.set_axon_ntff_profile_hook = set_axon_ntff_profile_hook
        mod.get_axon_ntff_profile_hook = get_axon_ntff_profile_hook
        sys.modules["antenv.axon_hooks"] = mod
        antenv.axon_hooks = mod
    except Exception:
        pass


_install_ntff_hook_shim()

F32 = mybir.dt.float32
BF16 = mybir.dt.bfloat16

B = 8          # batch == number of cores
C = 256        # channels
H = W = 64
L = H * W      # 4096 pixels
K2 = 9         # 3x3 patch
M = L * K2     # 36864 per-channel output length
MARG = 80      # input image margin (>= 73 needed)
OM = 8         # output tile margin (>= 8 needed)
QM = 8         # qs tile head margin (stretch group overhang, >= 8)
QTAIL = 580    # qs tile tail pad so run-zero rearrange views stay in-bounds
OFFS = [(kh - 1) * W + (kw - 1) for kh in range(3) for kw in range(3)]


def _ceil_div(a, b):
    return -(-a // b)


def _plan_qv_ops():
    """Per kk: (i_lo, i_hi, src_start, memsets) for qv[i] = Q_flat[9i+4].

    src position (relative to q image start at MARG) of qv[i] is
    src_start + 9*(i - i_lo).  memsets are (first, cnt, 64) runs in
    i-space where the query tap wraps an x-edge (must read as zero).
    """
    ops = []
    for kk in range(K2):
        s = L * kk
        i_lo = max(0, _ceil_div(s - 4, 9))
        i_hi = min(L, _ceil_div(s + L - 4, 9))
        src_start = 9 * i_lo + 4 - s + OFFS[kk]
        memsets = []
        kw = kk % 3
        if kw != 1:
            target = 0 if kw == 0 else 63
            i0 = (57 * (target - 4 + s)) % 64  # 57 = 9^-1 mod 64
            first = i_lo + ((i0 - i_lo) % 64)
            if first < i_hi:
                cnt = (i_hi - 1 - first) // 64 + 1
                memsets.append((first, cnt, 64))
        ops.append((i_lo, i_hi, src_start, memsets))
    return ops


def _plan_tt_ops():
    """Per kk: (g_lo, g_hi, ngroups, q0); TT covers l in [g_lo, g_hi)."""
    ops = []
    for kk in range(K2):
        s = L * kk
        g_lo = -(s % 9)
        g_hi = L + ((-(s + L)) % 9)
        ops.append((g_lo, g_hi, (g_hi - g_lo) // 9, (s + g_lo) // 9))
    return ops


QV_OPS = _plan_qv_ops()
TT_OPS = _plan_tt_ops()

# Units: chunks sharing one SBUF tile and one (two-queue) store. Chunk 0
# (broadcast, ACT-free) sits mid-schedule where its DVE TT fills the gap
# while ACT streams ahead; the schedule then ends on stretched pairs
# whose stores drain promptly. Modes per chunk instance: 's' = stretched
# (ACT builds qs, DVE 2x TT), 'b' = broadcast (DVE 1x TT).
UNITS = ((1, 2), (3, 4), (0,), (5, 6), (7, 8))
# (group, kk) pairs processed in broadcast mode — the DVE/ACT balance knob.
BCAST = frozenset({(0, 0), (1, 0), (0, 2), (1, 2)})
# Where to split the first stretched piece of group 0 (qv group offsets
# within source chunk 1) so its first stretch+TT pieces only need the
# first quarter / half of the query load.
HEAD_SPLITS = (118, 230)
# (group, kk) -> elems: chunk processed stretched up to the cut (which
# must be 9-aligned in global m) and broadcast past it. Fine-grained
# ACT/DVE balance: a half-chunk shifts ~2us of stretch off ACT for
# ~+1us of broadcast on DVE. kk must be the last stretched piece of its
# unit and have kw == 1 (no wrap masks across the cut).
PARTIAL = {(1, 4): 2048, (1, 6): 2046}
# Split the terminal piece's TT + store into halves so the drain tail
# overlaps compute (applies to this (group, kk)).
TAIL_SPLIT = (1, 8)


def build_graph(bcast=BCAST):
    nc = bacc.Bacc(None, target_bir_lowering=False)
    key_ext = nc.declare_dram_parameter("key_map", [C, L], BF16,
                                        isOutput=False)
    query_ext = nc.declare_dram_parameter("query_map", [C, L], BF16,
                                          isOutput=False)
    out_ext = nc.declare_dram_parameter("out", [C, M], BF16, isOutput=True)

    ngroups = C // 128
    with tile.TileContext(nc) as tc:
        with (
            tc.tile_pool(name="pads", bufs=1) as pads,
            tc.tile_pool(name="qvp", bufs=1) as qvp,
            tc.tile_pool(name="qsp", bufs=4) as qsp,
            tc.tile_pool(name="outs1", bufs=2) as outs1,
            tc.tile_pool(name="outs2", bufs=3) as outs2,
        ):
            key_pads, q_pads, qvs = [], [], []
            for g in range(ngroups):
                q_pad = pads.tile([128, MARG + L + MARG], BF16,
                                  name=f"q_pad{g}", tag=f"q_pad{g}")
                nc.vector.memset(q_pad[:, 0:MARG], 0.0)
                nc.vector.memset(q_pad[:, MARG + L:MARG + L + MARG], 0.0)
                key_pad = pads.tile([128, MARG + L + MARG], BF16,
                                    name=f"key_pad{g}", tag=f"key_pad{g}")
                nc.vector.memset(key_pad[:, 0:MARG], 0.0)
                nc.vector.memset(key_pad[:, MARG + L:MARG + L + MARG], 0.0)
                key_pads.append(key_pad)
                q_pads.append(q_pad)
                qv = qvp.tile([128, L], BF16, name=f"qv{g}", tag=f"qv{g}")
                qvs.append(qv)

            # Loads: all but one ride the sync queue (the sync sequencer
            # has nothing better to do than wait on the chain). The very
            # first key half goes on the scalar queue so it lands in
            # parallel with the first query half — ACT is idle then, so
            # the one inline wait costs nothing. Keeping the scalar queue
            # otherwise clear of loads is crucial: a chained load issue
            # sitting in the ACT instruction stream blocks the stretch
            # pipeline on load-completion semaphores.
            hL = L // 2
            nc.scalar.dma_start(key_pads[0][:, MARG:MARG + hL],
                                key_ext[0:128, 0:hL])
            # Chained so each load completes before the next starts:
            # unchained, all seven transfer concurrently and the FIRST
            # half arrives ~5x later, starving the early pipeline (and
            # the idle engines then also run at lower clocks).
            seq = [(0, "q", 0), (0, "q", 1), (0, "k", 1),
                   (1, "q", 0), (1, "k", 0), (1, "q", 1), (1, "k", 1)]
            prev_q = None
            for (g, t, h) in seq:
                pad = q_pads[g] if t == "q" else key_pads[g]
                ext = query_ext if t == "q" else key_ext
                qd = nc.sync.dma_start(
                    pad[:, MARG + h * hL:MARG + (h + 1) * hL],
                    ext[g * 128:(g + 1) * 128, h * hL:(h + 1) * hL])
                if prev_q is not None:
                    add_dep_helper(qd.ins, prev_q.ins, sync=True,
                                   reason="chain loads on sync queue")
                prev_q = qd

            def emit_qv_chunk(g, kk, j_lo=None, after=None):
                """DVE strided copies building the qv range broadcast chunk
                kk's TT reads: [s//9, ceil((s+L)/9)-1], which straddles into
                neighbor source chunks (+ query-wrap edge memsets, clipped
                per segment). j_lo override: partial chunks only need the
                post-cut range. after: pin the copies behind that op in
                the DVE stream — the scheduler otherwise hoists them ahead
                of earlier-ready TTs, stalling DVE on the full query load."""
                s = kk * L
                if j_lo is None:
                    j_lo = s // 9
                j_hi = _ceil_div(s + L, 9) - 1
                prev = after
                for kk2 in range(max(0, kk - 1), min(K2, kk + 2)):
                    i_lo, i_hi, src_start, msets = QV_OPS[kk2]
                    a, b = max(i_lo, j_lo), min(i_hi, j_hi + 1)
                    if a >= b:
                        continue
                    dst = qvs[g][:, a:b]
                    sa = MARG + src_start + 9 * (a - i_lo)
                    src = q_pads[g][:, sa:sa + 9 * (b - a):9]
                    op = nc.vector.tensor_copy(dst, src)
                    if prev is not None:
                        add_dep_helper(op.ins, prev.ins, sync=False,
                                       reason="qv build order")
                    prev = op
                    for (first, cnt, step) in msets:
                        j0 = max(0, _ceil_div(a - first, step))
                        j1 = (b - 1 - first) // step
                        if j0 > j1:
                            continue
                        f2 = first + j0 * step
                        c2 = j1 - j0 + 1
                        op = nc.vector.memset(
                            qvs[g][:, f2:f2 + (c2 - 1) * step + 1:step],
                            0.0)
                        add_dep_helper(op.ins, prev.ins, sync=False,
                                       reason="qv build order")
                        prev = op

            def emit_stretch(g, qs, s, wu, prev_act, splits=()):
                """ACT copies building qs[QM+x] = qv[(s+x)//9] for
                x in [0, wu), reading straight out of q_pad (the 9x fan-out
                amortizes the strided read; measured ~1.08 ns/elem).
                splits: qv group indices at which to break a copy so the
                early piece only depends on the first half-load. Returns
                the last ACT op (for stream-order chaining)."""
                i0 = s // 9
                i1 = (s + wu - 1) // 9
                for kk in range(K2):
                    i_lo, i_hi, src_start, _ = QV_OPS[kk]
                    a, b = max(i_lo, i0), min(i_hi, i1 + 1)
                    if a >= b:
                        continue
                    cuts = [a] + [c for c in splits if a < c < b] + [b]
                    for a2, b2 in zip(cuts[:-1], cuts[1:]):
                        dst = qs[:, QM + 9 * a2 - s:
                                 QM + 9 * b2 - s].rearrange(
                            "p (n k) -> p n k", k=9)
                        sa = MARG + src_start + 9 * (a2 - i_lo)
                        src = q_pads[g][:, sa:sa + 9 * (b2 - a2):9].unsqueeze(
                            2).broadcast_to([128, b2 - a2, 9])
                        op = nc.scalar.copy(dst, src)
                        if prev_act is not None:
                            add_dep_helper(op.ins, prev_act.ins, sync=False,
                                           reason="ACT stream order")
                        prev_act = op
                return prev_act

            def emit_qs_runzeros(qs, s, wu, kks, splits=()):
                """Zero 9-wide qs runs where the query tap wrapped an
                x-edge (stretched-chunk analogue of the qv edge memsets).
                splits mirror emit_stretch's so the early head piece's
                zeros don't depend on the late stretch copies."""
                i0 = s // 9
                i1 = (s + wu - 1) // 9
                for kk in kks:
                    i_lo, i_hi, _, msets = QV_OPS[kk]
                    a, b = max(i_lo, i0), min(i_hi, i1 + 1)
                    if a >= b:
                        continue
                    for (first, cnt, step) in msets:
                        cuts = [a] + [c for c in splits if a < c < b] + [b]
                        for a2, b2 in zip(cuts[:-1], cuts[1:]):
                            j0 = max(0, _ceil_div(a2 - first, step))
                            j1 = (b2 - 1 - first) // step
                            if j0 > j1:
                                continue
                            iz = first + j0 * step
                            cnt2 = j1 - j0 + 1
                            A = QM + 9 * iz - s
                            view = qs[:, A:A + 576 * cnt2].rearrange(
                                "p (n k) -> p n k", k=576)[:, :, 0:9]
                            nc.vector.memset(view, 0.0)

            prev_act = None
            prev_colmset = None
            pending_scalar = []
            qv_built = set()
            for g in range(ngroups):
                rows = slice(g * 128, (g + 1) * 128)
                key_pad, qv = key_pads[g], qvs[g]

                for iu, u in enumerate(UNITS):
                    wu = len(u) * L
                    s_u = u[0] * L
                    modes = ["p" if (g, kk) in PARTIAL else
                             ("b" if (g, kk) in bcast else "s")
                             for kk in u]
                    opool = outs1 if len(u) == 1 else outs2
                    ot = opool.tile([128, OM + wu + OM], BF16,
                                    name=f"ot{g}_{u[0]}",
                                    tag=f"ot{len(u)}")

                    # qs spans only the contiguous run of stretched pieces
                    qs = None
                    if "s" in modes or "p" in modes:
                        sidx = [i for i, m in enumerate(modes)
                                if m in ("s", "p")]
                        f_s, l_s = sidx[0], sidx[-1]
                        s_q = s_u + f_s * L
                        w_q = (l_s - f_s) * L + (
                            PARTIAL[(g, u[l_s])] if modes[l_s] == "p"
                            else L)
                        qs = qsp.tile([128, QM + w_q + QM + QTAIL], BF16,
                                      name=f"qs{g}_{u[0]}", tag="qs")
                        splits = ()
                        if g == 0 and iu == 0:
                            # break the first source at the half-image
                            # boundary (early TT on the first half-load)
                            # and peel the mid-unit straddle group so the
                            # first piece's second TT doesn't wait on the
                            # whole second source's stretch
                            splits = tuple(QV_OPS[u[f_s]][0] + h
                                           for h in HEAD_SPLITS) + (
                                QV_OPS[u[f_s]][1] + 1,)
                        prev_act = emit_stretch(g, qs, s_q, w_q, prev_act,
                                                splits)
                        emit_qs_runzeros(
                            qs, s_q, w_q,
                            [kk for kk, m in zip(u, modes)
                             if m in ("s", "p")],
                            splits)

                    # Deferred scalar-queue store halves of EARLIER units:
                    # emitted two units late, after this unit's stretch
                    # copies, so the ACT sequencer never stalls on a
                    # not-yet-ready data semaphore (one-unit deferral
                    # measured ~0.6-2.1us of ACT stall per store), yet
                    # the scalar DMA queue stays fed alongside sync.
                    # Before the terminal tail-split unit, flush
                    # everything so its inline quarter stores go last.
                    while pending_scalar:
                        nc.scalar.dma_start(*pending_scalar.pop(0))

                    prev_tt = None
                    for idx, (kk, mode) in enumerate(zip(u, modes)):
                        base = OM + idx * L
                        g_lo, g_hi, ng, q0 = TT_OPS[kk]
                        tail_split = (g, kk) == TAIL_SPLIT
                        if mode in ("s", "p"):
                            # flat contiguous bf16 TT -> DVE 2x packing;
                            # the group-0 head piece is split so TT-a only
                            # needs the first half-loads; the terminal
                            # piece is split so its store drains during
                            # the second half's compute.
                            qb = QM + (idx - f_s) * L
                            send = PARTIAL.get((g, kk), L)
                            pieces = [(0, send)]
                            if g == 0 and iu == 0 and idx == f_s:
                                cuts = [9 * h - (9 - kk)
                                        for h in HEAD_SPLITS]
                                edges = [0] + cuts + [send]
                                pieces = list(zip(edges[:-1], edges[1:]))
                            elif tail_split:
                                pieces = [(0, send // 2), (send // 2, send)]
                            for (xa, xb) in pieces:
                                tt = nc.vector.tensor_mul(
                                    ot[:, base + xa:base + xb],
                                    key_pad[:, MARG + OFFS[kk] + xa:
                                            MARG + OFFS[kk] + xb],
                                    qs[:, qb + xa:qb + xb])
                                if prev_tt is not None:
                                    add_dep_helper(
                                        tt.ins, prev_tt.ins, sync=False,
                                        reason="piece order in tile")
                                prev_tt = tt
                                if tail_split:
                                    kw = kk % 3
                                    if kw != 1:
                                        c00 = 0 if kw == 0 else 63
                                        first = xa + ((c00 - xa) % 64)
                                        prev_tt = nc.vector.memset(
                                            ot[:, base + first:
                                               base + xb:64], 0.0)
                                    deng = nc.sync if xa == 0 else nc.scalar
                                    deng.dma_start(
                                        out_ext[rows,
                                                u[0] * L + idx * L + xa:
                                                u[0] * L + idx * L + xb],
                                        ot[:, base + xa:base + xb])
                            if mode == "p":
                                # broadcast remainder past the cut (send
                                # is 9-aligned in global m by choice)
                                q0r = (kk * L + send) // 9
                                ngr = (g_hi - send) // 9
                                emit_qv_chunk(g, kk, j_lo=q0r,
                                              after=prev_tt)
                                dst = ot[:, base + send:
                                         base + g_hi].rearrange(
                                    "p (n k) -> p n k", k=9)
                                src_k = key_pad[
                                    :, MARG + send + OFFS[kk]:
                                    MARG + g_hi + OFFS[kk]].rearrange(
                                    "p (n k) -> p n k", k=9)
                                src_q = qv[:, q0r:q0r + ngr].unsqueeze(
                                    2).broadcast_to([128, ngr, 9])
                                tt = nc.vector.tensor_mul(dst, src_k, src_q)
                                add_dep_helper(tt.ins, prev_tt.ins,
                                               sync=False,
                                               reason="piece order in tile")
                                prev_tt = tt
                        else:
                            if (g, kk) not in qv_built:
                                emit_qv_chunk(g, kk, after=prev_tt)
                                qv_built.add((g, kk))
                            if idx > 0:
                                # boundary group: first p outputs share
                                # qv[q0]; emit with a tiny broadcast TT
                                p = 9 - kk
                                nc.vector.tensor_mul(
                                    ot[:, base:base + p],
                                    key_pad[:, MARG + OFFS[kk]:
                                            MARG + OFFS[kk] + p],
                                    qv[:, q0:q0 + 1].broadcast_to([128, p]))
                                g_lo, q0, ng = p, q0 + 1, ng - 1
                            dst = ot[:, base + g_lo:base + g_hi].rearrange(
                                "p (n k) -> p n k", k=9)
                            src_k = key_pad[:, MARG + g_lo + OFFS[kk]:
                                            MARG + g_hi + OFFS[kk]].rearrange(
                                "p (n k) -> p n k", k=9)
                            src_q = qv[:, q0:q0 + ng].unsqueeze(
                                2).broadcast_to([128, ng, 9])
                            tt = nc.vector.tensor_mul(dst, src_k, src_q)
                            if prev_tt is not None:
                                add_dep_helper(tt.ins, prev_tt.ins,
                                               sync=False,
                                               reason="piece order in tile")
                            prev_tt = tt
                        if prev_colmset is not None:
                            add_dep_helper(tt.ins, prev_colmset.ins,
                                           sync=False,
                                           reason="colmset before next TT")
                            prev_colmset = None

                        kw = kk % 3
                        if not tail_split:
                            if kw == 0:
                                prev_colmset = nc.vector.memset(
                                    ot[:, base:base + L:64], 0.0)
                                prev_tt = prev_colmset
                            elif kw == 2:
                                prev_colmset = nc.vector.memset(
                                    ot[:, base + 63:base + L:64], 0.0)
                                prev_tt = prev_colmset

                        # Store as two halves on BOTH HWDGE queues: one
                        # queue alone tops out ~240 GB/s, two concurrently
                        # ~424. The sync half (= first piece for pairs)
                        # issues as soon as that piece's data is final;
                        # the scalar half is deferred into the next unit's
                        # emission point (see pending_scalar above).
                        hw = wu // 2
                        c0 = u[0] * L
                        if idx == (0 if len(u) > 1 else len(u) - 1):
                            nc.sync.dma_start(out_ext[rows, c0:c0 + hw],
                                              ot[:, OM:OM + hw])
                    if (g, u[-1]) != TAIL_SPLIT:
                        pending_scalar.append(
                            (out_ext[rows, c0 + hw:c0 + wu],
                             ot[:, OM + hw:OM + wu]))
            for ps in pending_scalar:
                nc.scalar.dma_start(*ps)
    nc.compile()
    return nc


_GRAPH_CACHE = {}


def _get_graph():
    if "nc" not in _GRAPH_CACHE:
        _GRAPH_CACHE["nc"] = build_graph()
    return _GRAPH_CACHE["nc"]


def kernel(key_map: np.ndarray, query_map: np.ndarray,
           _trace: bool = False, _tmpdir: str | None = None):
    import ml_dtypes
    bf16 = ml_dtypes.bfloat16
    key_map = np.ascontiguousarray(key_map, dtype=np.float32).astype(bf16)
    query_map = np.ascontiguousarray(query_map, dtype=np.float32).astype(bf16)
    assert key_map.shape == (B, C, H, W), key_map.shape

    nc = _get_graph()
    in_maps = [
        {"key_map": key_map[b].reshape(C, L),
         "query_map": query_map[b].reshape(C, L)}
        for b in range(B)
    ]
    res = run_bass_kernel_spmd(
        nc, in_maps, core_ids=list(range(B)),
        trace=_trace, tmpdir=_tmpdir,
    )
    out = np.stack([res.results[b]["out"] for b in range(B)])
    _GRAPH_CACHE["last_exec_time_ns"] = res.exec_time_ns
    _GRAPH_CACHE["last_results"] = res
    return out.astype(np.float32).reshape(B, C, L, K2)

